# revision 6
# baseline (speedup 1.0000x reference)
"""Trainium2 Bass kernel for nn_Attention_27977416966318 (sparse_attention).

score[b,s] = v . tanh(W @ concat(static[b,s], dynamic[b,s], dec[b]))
out = softmax(score, axis=1)

Shapes: static/dynamic [64, 2048, 256] f32, decoder_hidden [64, 256],
v [1, 768], W [768, 768].  Output [64, 2048] f32.

Strategy: data-parallel over batch B=64 across 8 NeuronCores (8 batches
per core).  W @ cat decomposes as W1 @ static + W2 @ dynamic + (W3 @ dec[b])
where the last term is a per-batch bias computed once.  Inputs are
pre-transposed on the host to feature-major [256, tokens] layout so the
contraction dim lands on SBUF partitions.  Matmuls run as float32r
(full-rate fp32).  tanh+bias fused on the scalar engine reading PSUM.
The v-dot is a [128,1]-stationary matmul accumulated over the 6 output
tiles.  Softmax over S stays local per core (8x2048 tile).
"""

import os

import numpy as np

import concourse.bass as bass
from concourse import bacc
import concourse.mybir as mybir
import concourse.tile as tile
from concourse.bass_utils import run_bass_kernel_spmd

B, S, H = 64, 2048, 256
H3 = 3 * H          # 768
NCORES = 8
BL = B // NCORES    # 8 batches per core
T = BL * S          # 16384 tokens per core
KT = 4              # contraction k-tiles of 128 (2 static + 2 dynamic)
MT = H3 // 128      # 6 output o-tiles
GT = 1024           # tokens per group (2 chunks of 512)
NG = T // GT        # 16 groups per core
F32 = mybir.dt.float32
F32R = mybir.dt.float32r
TANH = mybir.ActivationFunctionType.Tanh
EXP = mybir.ActivationFunctionType.Exp

_CACHED = {}


def build_bass():
    nc = bacc.Bacc(None, target_bir_lowering=False, debug=False)
    xs = nc.dram_tensor("xs_t", [H, T], F32R, kind="ExternalInput")
    xd = nc.dram_tensor("xd_t", [H, T], F32R, kind="ExternalInput")
    dec = nc.dram_tensor("dec_t", [H, BL], F32R, kind="ExternalInput")
    wt = nc.dram_tensor("wt", [H3, H3], F32R, kind="ExternalInput")
    vv = nc.dram_tensor("v", [1, H3], F32R, kind="ExternalInput")
    out = nc.dram_tensor("out", [BL, S], F32, kind="ExternalOutput")

    with tile.TileContext(nc) as tc:
        with (
            tc.tile_pool(name="const", bufs=1) as constp,
            tc.tile_pool(name="xp", bufs=2) as xp,
            tc.tile_pool(name="thp", bufs=3) as thp,
            tc.tile_pool(name="misc", bufs=1) as miscp,
            tc.tile_pool(name="hps", bufs=3, space="PSUM") as hps,
            tc.tile_pool(name="sps", bufs=2, space="PSUM") as sps,
        ):
            # ---- constants ----
            # wt is W.T: [k=cat-feature, o].  k-tiles 0-1 static, 2-3 dynamic,
            # 4-5 decoder.
            wt_sb = constp.tile([128, 2 * KT - 2, H3], F32R)  # [128, 6, 768]
            for t in range(6):
                nc.sync.dma_start(
                    out=wt_sb[:, t, :], in_=wt[t * 128 : (t + 1) * 128, :]
                )
            v_sb = constp.tile([128, MT], F32R)
            nc.sync.dma_start(
                out=v_sb, in_=vv[0].rearrange("(t p) -> p t", p=128)
            )
            dec_sb = constp.tile([128, 2, BL], F32R)
            nc.sync.dma_start(
                out=dec_sb, in_=dec.rearrange("(t p) b -> p t b", p=128)
            )

            # ---- per-batch bias: bias[o, b] = sum_k W3T[k, o] dec[k, b] ----
            bias_sb = constp.tile([128, MT, BL], F32)
            for m in range(MT):
                bias_ps = sps.tile([128, BL], F32, tag="s", name=f"bias_ps_{m}")
                for i in range(2):
                    nc.tensor.matmul(
                        out=bias_ps,
                        lhsT=wt_sb[:, 4 + i, m * 128 : (m + 1) * 128],
                        rhs=dec_sb[:, i, :],
                        start=(i == 0),
                        stop=(i == 1),
                    )
                nc.vector.tensor_copy(out=bias_sb[:, m, :], in_=bias_ps)

            scores_sb = miscp.tile([BL, S], F32)

            # ---- main loop ----
            for b in range(BL):
                stage = miscp.tile([1, S], F32, tag="stage", bufs=2, name=f"stage_{b}")
                score_ps = [None] * 4
                for g in range(2):
                    tok0 = b * S + g * GT
                    xt = []
                    for kt in range(KT):
                        src = xs if kt < 2 else xd
                        r0 = (kt % 2) * 128
                        x_tile = xp.tile(
                            [128, GT], F32R, tag=f"x{kt}", name=f"x_{b}_{g}_{kt}"
                        )
                        nc.sync.dma_start(
                            out=x_tile, in_=src[r0 : r0 + 128, tok0 : tok0 + GT]
                        )
                        xt.append(x_tile)
                    for m in range(MT):
                        h_ps = hps.tile([128, GT], F32, tag="h", name=f"h_{b}_{g}_{m}")
                        for kt in range(KT):
                            for c in range(2):
                                nc.tensor.matmul(
                                    out=h_ps[:, c * 512 : (c + 1) * 512],
                                    lhsT=wt_sb[
                                        :, kt, m * 128 : (m + 1) * 128
                                    ],
                                    rhs=xt[kt][:, c * 512 : (c + 1) * 512],
                                    start=(kt == 0),
                                    stop=(kt == KT - 1),
                                )
                        th = thp.tile([128, GT], F32R, tag="tanh", name=f"th_{b}_{g}_{m}")
                        nc.scalar.activation(
                            out=th, in_=h_ps, func=TANH, bias=bias_sb[:, m, b : b + 1]
                        )
                        for c in range(2):
                            ci = g * 2 + c
                            if m == 0:
                                score_ps[ci] = sps.tile(
                                    [1, 512], F32, tag="s", name=f"sc_{b}_{ci}"
                                )
                            nc.tensor.matmul(
                                out=score_ps[ci],
                                lhsT=v_sb[:, m : m + 1],
                                rhs=th[:, c * 512 : (c + 1) * 512],
                                start=(m == 0),
                                stop=(m == MT - 1),
                            )
                    for c in range(2):
                        ci = g * 2 + c
                        nc.vector.tensor_copy(
                            out=stage[:, ci * 512 : (ci + 1) * 512],
                            in_=score_ps[ci],
                        )
                # SBUF->SBUF DMA moves the row to partition b
                nc.sync.dma_start(out=scores_sb[b : b + 1, :], in_=stage)

            # ---- softmax over S per batch row ----
            mx = miscp.tile([BL, 1], F32)
            nc.vector.tensor_reduce(
                out=mx, in_=scores_sb, axis=mybir.AxisListType.X,
                op=mybir.AluOpType.max, negate=True,
            )
            ex = miscp.tile([BL, S], F32)
            nc.scalar.activation(out=ex, in_=scores_sb, func=EXP, bias=mx)
            sm = miscp.tile([BL, 1], F32)
            nc.vector.reduce_sum(out=sm, in_=ex, axis=mybir.AxisListType.X)
            rs = miscp.tile([BL, 1], F32)
            nc.vector.reciprocal(out=rs, in_=sm)
            ob = miscp.tile([BL, S], F32)
            nc.vector.tensor_scalar_mul(out=ob, in0=ex, scalar1=rs)
            nc.sync.dma_start(out=out[:, :], in_=ob)

    nc.compile()
    return nc


def kernel(static, dynamic, decoder_hidden, v, W):
    static = np.ascontiguousarray(np.asarray(static, dtype=np.float32))
    dynamic = np.ascontiguousarray(np.asarray(dynamic, dtype=np.float32))
    decoder_hidden = np.ascontiguousarray(np.asarray(decoder_hidden, dtype=np.float32))
    v = np.ascontiguousarray(np.asarray(v, dtype=np.float32))
    W = np.ascontiguousarray(np.asarray(W, dtype=np.float32))

    wt = np.ascontiguousarray(W.T)
    in_maps = []
    for c in range(NCORES):
        sl = slice(c * BL, (c + 1) * BL)
        xs_t = np.ascontiguousarray(static[sl].reshape(T, H).T)
        xd_t = np.ascontiguousarray(dynamic[sl].reshape(T, H).T)
        dec_t = np.ascontiguousarray(decoder_hidden[sl].T)
        in_maps.append(
            {"xs_t": xs_t, "xd_t": xd_t, "dec_t": dec_t, "wt": wt, "v": v}
        )

    if "nc" not in _CACHED:
        _CACHED["nc"] = build_bass()
    nc = _CACHED["nc"]

    trace = bool(int(os.environ.get("KERNEL_TRACE", "0")))
    res = run_bass_kernel_spmd(
        nc, in_maps, core_ids=list(range(NCORES)), trace=trace,
        trace_cores=list(range(NCORES)) if trace else None,
    )
    _CACHED["last_result"] = res

    out = np.concatenate([r["out"] for r in res.results], axis=0)
    return out


# revision 9
# speedup vs baseline: 1.1341x; 1.1341x over previous
"""Trainium2 Bass kernel for nn_Attention_27977416966318 (sparse_attention).

score[b,s] = v . tanh(W @ concat(static[b,s], dynamic[b,s], dec[b]))
out = softmax(score, axis=1)

Shapes: static/dynamic [64, 2048, 256] f32, decoder_hidden [64, 256],
v [1, 768], W [768, 768].  Output [64, 2048] f32.

Strategy: data-parallel over batch B=64 across 8 NeuronCores (8 batches
per core).  W @ cat decomposes as W1 @ static + W2 @ dynamic + (W3 @ dec[b])
where the last term is a per-batch bias computed once on-device.  Inputs
are pre-transposed on the host to feature-major [256, tokens] layout so
the contraction dim lands on SBUF partitions.  Matmuls run as float32r
(full-rate fp32).  tanh+bias fused on the scalar engine reading PSUM.
The v-dot runs as 4 column-packed (tile_position) M=1 matmuls that
execute concurrently on disjoint 32-column strips of the PE array.
exp() is fused into the score PSUM->SBUF copies (scores are bounded, so
the max-free softmax is numerically safe); softmax denominator+scale per
core on an [8, 2048] tile.
"""

import os

import numpy as np

import concourse.bass as bass
from concourse import bacc
import concourse.mybir as mybir
import concourse.tile as tile
from concourse.bass_utils import run_bass_kernel_spmd

B, S, H = 64, 2048, 256
H3 = 3 * H          # 768
NCORES = 8
BL = B // NCORES    # 8 batches per core
T = BL * S          # 16384 tokens per core
KT = 4              # contraction k-tiles of 128 (2 static + 2 dynamic)
MT = H3 // 128      # 6 output o-tiles
GT = 1024           # tokens per group (2 chunks of 512)
F32 = mybir.dt.float32
F32R = mybir.dt.float32r
BF16 = mybir.dt.bfloat16
TANH = mybir.ActivationFunctionType.Tanh
EXP = mybir.ActivationFunctionType.Exp

_CACHED = {}


def build_bass():
    nc = bacc.Bacc(None, target_bir_lowering=False, debug=False)
    xs = nc.dram_tensor("xs_t", [H, T], F32R, kind="ExternalInput")
    xd = nc.dram_tensor("xd_t", [H, T], F32R, kind="ExternalInput")
    dec = nc.dram_tensor("dec_t", [H, BL], F32R, kind="ExternalInput")
    wt = nc.dram_tensor("wt", [H3, H3], F32R, kind="ExternalInput")
    vv = nc.dram_tensor("v", [1, H3], F32, kind="ExternalInput")
    out = nc.dram_tensor("out", [BL, S], F32, kind="ExternalOutput")

    with tile.TileContext(nc) as tc:
        with (
            tc.tile_pool(name="const", bufs=1) as constp,
            tc.tile_pool(name="xp", bufs=2) as xp,
            tc.tile_pool(name="thp", bufs=13) as thp,
            tc.tile_pool(name="misc", bufs=1) as miscp,
            tc.tile_pool(name="hps", bufs=3, space="PSUM") as hps,
            tc.tile_pool(name="sps", bufs=2, space="PSUM") as sps,
        ):
            # ---- first x tiles on the sync queue (issue ASAP) ----
            first_xt = []
            for kt_i in range(KT):
                src = xs if kt_i < 2 else xd
                r0 = (kt_i % 2) * 128
                x_tile = xp.tile([128, GT], F32R, tag=f"x{kt_i}", name=f"x_0_0_{kt_i}")
                nc.sync.dma_start(out=x_tile, in_=src[r0 : r0 + 128, 0:GT])
                first_xt.append(x_tile)

            # ---- constants on the scalar (HWDGE) queue, in parallel ----
            # wt is W.T: [k=cat-feature, o].  k-tiles 0-1 static, 2-3 dynamic,
            # 4-5 decoder.
            wt_sb = constp.tile([128, 6, H3], F32R)
            for t in range(6):
                nc.scalar.dma_start(
                    out=wt_sb[:, t, :], in_=wt[t * 128 : (t + 1) * 128, :]
                )
            # v in bf16: the v-dot runs as a bf16 matmul (col-packable)
            v_sb = constp.tile([128, MT], BF16)
            nc.gpsimd.dma_start(out=v_sb, in_=vv[0].rearrange("(t p) -> p t", p=128))
            dec_sb = constp.tile([128, 2, BL], F32R)
            nc.scalar.dma_start(
                out=dec_sb, in_=dec.rearrange("(t p) b -> p t b", p=128)
            )

            # ---- per-batch bias: bias[o, b] = sum_k W3T[k, o] dec[k, b] ----
            bias_sb = constp.tile([128, MT, BL], F32)
            for m in range(MT):
                bias_ps = sps.tile([128, BL], F32, tag="s", name=f"bias_ps_{m}")
                for i in range(2):
                    nc.tensor.matmul(
                        out=bias_ps,
                        lhsT=wt_sb[:, 4 + i, m * 128 : (m + 1) * 128],
                        rhs=dec_sb[:, i, :],
                        start=(i == 0),
                        stop=(i == 1),
                    )
                nc.vector.tensor_copy(out=bias_sb[:, m, :], in_=bias_ps)

            escores = miscp.tile([BL, S], F32)  # exp(score), filled per b

            # ---- main loop ----
            for b in range(BL):
                # one PSUM bank holds the 4 chunk scores on partitions
                # 0/32/64/96 (column-group packing)
                score_ps = sps.tile([128, 512], F32, tag="s", name=f"sa_{b}")
                ths = {}
                for g in range(2):
                    tok0 = b * S + g * GT
                    if b == 0 and g == 0:
                        xt = first_xt
                    else:
                        xt = []
                        for kt_i in range(KT):
                            src = xs if kt_i < 2 else xd
                            r0 = (kt_i % 2) * 128
                            x_tile = xp.tile(
                                [128, GT], F32R, tag=f"x{kt_i}",
                                name=f"x_{b}_{g}_{kt_i}",
                            )
                            nc.sync.dma_start(
                                out=x_tile, in_=src[r0 : r0 + 128, tok0 : tok0 + GT]
                            )
                            xt.append(x_tile)
                    for m in range(MT):
                        h_ps = hps.tile([128, GT], F32, tag="h", name=f"h_{b}_{g}_{m}")
                        for kt_i in range(KT):
                            for c in range(2):
                                nc.tensor.matmul(
                                    out=h_ps[:, c * 512 : (c + 1) * 512],
                                    lhsT=wt_sb[:, kt_i, m * 128 : (m + 1) * 128],
                                    rhs=xt[kt_i][:, c * 512 : (c + 1) * 512],
                                    start=(kt_i == 0),
                                    stop=(kt_i == KT - 1),
                                )
                        th = thp.tile(
                            [128, GT], BF16, tag="tanh", name=f"th_{b}_{g}_{m}"
                        )
                        nc.scalar.activation(
                            out=th, in_=h_ps, func=TANH, bias=bias_sb[:, m, b : b + 1]
                        )
                        ths[(g, m)] = th
                # column-packed v-dot: 4 chunks concurrently on col strips
                for m in range(MT):
                    for ci in range(4):
                        gg, cc = divmod(ci, 2)
                        nc.tensor.matmul(
                            out=score_ps[32 * ci : 32 * ci + 1, :],
                            lhsT=v_sb[:, m : m + 1],
                            rhs=ths[(gg, m)][:, cc * 512 : (cc + 1) * 512],
                            start=(m == 0),
                            stop=(m == MT - 1),
                            tile_position=(0, 32 * ci),
                        )
                # exp fused into the PSUM->SBUF copies (partition-aligned)
                stage = miscp.tile(
                    [128, 512], F32, tag="stage", bufs=2, name=f"stage_{b}"
                )
                for ci in range(4):
                    nc.scalar.activation(
                        out=stage[32 * ci : 32 * ci + 1, :],
                        in_=score_ps[32 * ci : 32 * ci + 1, :],
                        func=EXP,
                    )
                # gather rows {0,32,64,96} -> escores[b] via SBUF->SBUF DMA
                nc.gpsimd.dma_start(
                    out=escores[b : b + 1, :],
                    in_=stage.rearrange("(c r) f -> c r f", c=4)[:, 0, :],
                )

            # ---- softmax denominator + scale ----
            sm = miscp.tile([BL, 1], F32)
            nc.vector.reduce_sum(out=sm, in_=escores, axis=mybir.AxisListType.X)
            rs = miscp.tile([BL, 1], F32)
            nc.vector.reciprocal(out=rs, in_=sm)
            ob = miscp.tile([BL, S], F32)
            nc.vector.tensor_scalar_mul(out=ob, in0=escores, scalar1=rs)
            nc.sync.dma_start(out=out[:, :], in_=ob)

    nc.compile()
    return nc


def kernel(static, dynamic, decoder_hidden, v, W):
    static = np.ascontiguousarray(np.asarray(static, dtype=np.float32))
    dynamic = np.ascontiguousarray(np.asarray(dynamic, dtype=np.float32))
    decoder_hidden = np.ascontiguousarray(np.asarray(decoder_hidden, dtype=np.float32))
    v = np.ascontiguousarray(np.asarray(v, dtype=np.float32))
    W = np.ascontiguousarray(np.asarray(W, dtype=np.float32))

    wt = np.ascontiguousarray(W.T)
    in_maps = []
    for c in range(NCORES):
        sl = slice(c * BL, (c + 1) * BL)
        xs_t = np.ascontiguousarray(static[sl].reshape(T, H).T)
        xd_t = np.ascontiguousarray(dynamic[sl].reshape(T, H).T)
        dec_t = np.ascontiguousarray(decoder_hidden[sl].T)
        in_maps.append(
            {"xs_t": xs_t, "xd_t": xd_t, "dec_t": dec_t, "wt": wt, "v": v}
        )

    if "nc" not in _CACHED:
        _CACHED["nc"] = build_bass()
    nc = _CACHED["nc"]

    trace = bool(int(os.environ.get("KERNEL_TRACE", "0")))
    res = run_bass_kernel_spmd(
        nc, in_maps, core_ids=list(range(NCORES)), trace=trace,
        trace_cores=list(range(NCORES)) if trace else None,
    )
    _CACHED["last_result"] = res

    out = np.concatenate([r["out"] for r in res.results], axis=0)
    return out
